# revision 35
# baseline (speedup 1.0000x reference)
"""Causal linear attention (fast-transformers style) on 8 Trainium2 NeuronCores.

Full inputs in, full output out. Sharding: the 32 (n, h) pairs are split
8 ways -> each core owns 4 pairs (one batch n, 4 adjacent heads); the
per-(n,h) cumulative KV state never crosses cores (no collectives).

v3 design (measured-rate driven; baseline was 114us, v2 104us):
- Host pre-casts to bf16 and fuses q|k|v' into ONE dram tensor per core
  (v' = [v*kl, kl] per pair: key_lengths folded into the value/ones
  columns on the host; the causal mask is then a plain multiply).
- 4-chunk DMA loads / stores; host does the final num/den divide.
- PE transposes via is_transpose -> bf16 PSUM (evac at DVE 2x rate).
- Uniform padded layout [q0|Z][q1|Z][q2|Z][q3|Z]: one 3-dim strided STT
  covers all four q blocks of a chunk; the S state lives at partitions
  0:64 for every pair, with the upper half of the s_sb tiles zeroed once
  so the 128-partition inter matmuls read zeros there.
- Software pipeline tuned against the measured loop-carried chain:
  the DVE queue runs [mask(2m), tq-evac(f), max, mask(2m+1), phi] so the
  attn->mask->...->tq->attn cycle is short; PE runs S-updates first
  (unblocks s_evac), staggers attn per chunk (attn(f,0) mid-iteration so
  mask(2f) next iteration has slack).
- Engine split: ACT {exp, tk-evac, s_evac, out-evac}, DVE {masks,
  tq-evac, max, phi}, PE {matmuls}. gpsimd only memsets (measured 40x
  slower than DVE on bulk elementwise).

Per-core math (chunked scan, C=128 rows, pairs j=0..3):
  phi(x) = elu(x)+1 = max(x,0) + min(exp(x),1)      (exact identity)
  per chunk, per pair j:
    attn_T[d,c] = sum_e K[d,e] Q[c,e], masked to d<=c (triu in [d,c])
    out = attn_T^T @ V' + Q @ S        (V' = [v*kl, kl], 65 cols)
    S  += K^T @ V'                     (PSUM accumulation)
  host: result = out[:, :64] / out[:, 64]
"""

from contextlib import ExitStack

import ml_dtypes
import numpy as np

import concourse.bacc as bacc
import concourse.mybir as mybir
import concourse.tile as tile
from concourse.bass_utils import run_bass_kernel_spmd

F32 = mybir.dt.float32
BF16 = mybir.dt.bfloat16
AF = mybir.ActivationFunctionType
ALU = mybir.AluOpType

N, L, H, E = 4, 4096, 8, 64
P = 4              # (n,h) pairs per core
C = 128            # chunk rows
M1 = E + 1         # v cols + kl column (denominator)
W = 3 * P * E + P  # fused qkv row: q 256 | k 256 | v' 260
N_CORES = 8
NCH = L // C       # 32 chunks
NPAIR = NCH // 2   # 16 chunk-pairs (pipeline iterations)
NG = NCH // 4      # 8 dma groups


def build_core_kernel(nc):
    qkv_d = nc.dram_tensor("qkv", [L, W], BF16, kind="ExternalInput").ap()
    tril_d = nc.dram_tensor("tril", [C, C], BF16, kind="ExternalInput").ap()
    ident_d = nc.dram_tensor("ident", [C, C], BF16, kind="ExternalInput").ap()
    out_d = nc.dram_tensor("out", [L, P * M1], BF16, kind="ExternalOutput").ap()

    qkvr = qkv_d.rearrange("(g t p) x -> g p t x", t=4, p=C)
    outr = out_d.rearrange("(g t p) x -> g p t x", t=4, p=C)

    with tile.TileContext(nc) as tc, ExitStack() as ctx:
        consts = ctx.enter_context(tc.tile_pool(name="consts", bufs=1))
        qkv_pool = ctx.enter_context(tc.tile_pool(name="qkv", bufs=1))
        e_pool = ctx.enter_context(tc.tile_pool(name="exp", bufs=1))
        x_pool = ctx.enter_context(tc.tile_pool(name="xmax", bufs=1))
        phi_pool = ctx.enter_context(tc.tile_pool(name="phi", bufs=1))
        qkT_pool = ctx.enter_context(tc.tile_pool(name="qkT", bufs=1))
        attn_pool = ctx.enter_context(tc.tile_pool(name="attn", bufs=1))
        osb_pool = ctx.enter_context(tc.tile_pool(name="osb", bufs=1))
        ssb_pool = ctx.enter_context(tc.tile_pool(name="ssb", bufs=1))
        ps_s = ctx.enter_context(tc.tile_pool(name="psS", bufs=1, space="PSUM"))
        ps_tq = ctx.enter_context(tc.tile_pool(name="psTq", bufs=1, space="PSUM"))
        ps_attn = ctx.enter_context(tc.tile_pool(name="psA", bufs=1, space="PSUM"))
        ps_out = ctx.enter_context(tc.tile_pool(name="psO", bufs=1, space="PSUM"))

        tril_t = consts.tile([C, C], BF16)
        nc.scalar.dma_start(out=tril_t[:], in_=tril_d[:])
        ident = consts.tile([C, C], BF16)
        nc.scalar.dma_start(out=ident[:], in_=ident_d[:])

        # rings (explicit tiles; slot = index % depth)
        qkv_bufs = [qkv_pool.tile([C, 4, W], BF16, name=f"qkv{i}") for i in range(4)]
        et_bufs = [e_pool.tile([C, 4, 2 * P * E], BF16, name=f"et{i}") for i in range(2)]
        xm_bufs = [x_pool.tile([C, 4, 2 * P * E], BF16, name=f"xm{i}") for i in range(2)]
        # phi: compact per chunk-pair [C, 2, 512] = [q0 q1 q2 q3 | k0 k1 k2 k3]
        # (no padding: matmul operands use 64-partition slices, all base 0)
        phi_bufs = [phi_pool.tile([C, 2, 4 * P * E // 2], BF16, name=f"phi{i}")
                    for i in range(4)]
        # transposed q+k: per-pair [64, 128] blocks, q at 0-3, k at 4-7
        qkT_bufs = [qkT_pool.tile([64, 2, 8 * C], BF16, name=f"qkT{i}") for i in range(3)]
        attn_bufs = [attn_pool.tile([C, 2, P * C], BF16, name=f"asb{i}") for i in range(2)]
        osb_bufs = [osb_pool.tile([C, 4, P * M1], BF16, name=f"osb{i}") for i in range(2)]
        ssb_bufs = [ssb_pool.tile([64, P * M1], BF16, name=f"ssb{i}") for i in range(2)]

        s_psum = ps_s.tile([C, 512], F32)
        # per-chunk transposed q+k tiles, ping-ponged by chunk parity
        tqk_ps = [ps_tq.tile([64, 8 * C], BF16, name=f"tqk{i}") for i in range(2)]
        # per-chunk attn tiles, ping-ponged by chunk parity
        attn_ps = [ps_attn.tile([C, P * C], F32, name=f"aps{i}") for i in range(2)]
        # one full bank per chunk so the j-slices never straddle a bank
        out_ps = ps_out.tile([C, 2, 512], F32)

        # ---- stage helpers (p = chunk-pair index, ci = chunk index) ----
        def qk_slice(p):
            return qkv_bufs[(p // 2) % 4][:, (p % 2) * 2 : (p % 2) * 2 + 2, 0 : 2 * P * E]

        def vx(ci):
            t = ci % 4
            return qkv_bufs[(ci // 4) % 4][:, t : t + 1, 2 * P * E : W].rearrange(
                "p a x -> p (a x)"
            )

        def load(g):
            nc.sync.dma_start(out=qkv_bufs[g % 4][:], in_=qkvr[g])

        def store(g):
            nc.sync.dma_start(out=outr[g], in_=osb_bufs[g % 2][:])

        def exp_(g):
            # whole 4-chunk load group in one activation
            nc.scalar.activation(
                et_bufs[g % 2][:], qkv_bufs[g % 4][:, :, 0 : 2 * P * E], AF.Exp
            )

        def max_(g):
            nc.vector.tensor_scalar_max(
                xm_bufs[g % 2][:], qkv_bufs[g % 4][:, :, 0 : 2 * P * E], 0.0
            )

        def phi_(p):
            # phi = min(exp,1) + max(x,0): one fused STT (1x rate but a
            # single instruction and no extra cross-engine edges)
            o = (p % 2) * 2
            nc.vector.scalar_tensor_tensor(
                phi_bufs[p % 4][:],
                et_bufs[(p // 2) % 2][:, o : o + 2, :], 1.0,
                xm_bufs[(p // 2) % 2][:, o : o + 2, :],
                op0=ALU.min, op1=ALU.add,
            )

        def transp(p, t):
            # per-pair [128, 64] -> [64, 128] transposes, all at base 0
            phi = phi_bufs[p % 4]
            for b in range(8):
                nc.tensor.transpose(
                    tqk_ps[t][:, b * C : (b + 1) * C],
                    phi[:, t, b * E : (b + 1) * E], ident[:],
                )

        def tqk_evac(p, t):
            dst = qkT_bufs[p % 3][:, t : t + 1, :].rearrange("p a x -> p (a x)")
            if t == 0:
                nc.vector.tensor_copy(dst, tqk_ps[t][:])
            else:
                nc.scalar.activation(dst, tqk_ps[t][:], AF.Copy)

        def qT(p, t, j):
            return qkT_bufs[p % 3][:, t, j * C : (j + 1) * C]

        def kT(p, t, j):
            return qkT_bufs[p % 3][:, t, (4 + j) * C : (5 + j) * C]

        def attn(p, t):
            ci = 2 * p + t
            for j in range(P):
                nc.tensor.matmul(
                    attn_ps[ci % 2][:, j * C : (j + 1) * C],
                    kT(p, t, j), qT(p, t, j),
                    start=(j == 0), stop=(j == P - 1),
                )

        def mask(ci):
            t = ci % 2
            nc.vector.tensor_mul(
                attn_bufs[(ci // 2) % 2][:, t : t + 1, :].rearrange(
                    "p a (j c) -> p (a j) c", c=C
                ),
                attn_ps[ci % 2][:].rearrange("p (j c) -> p j c", c=C),
                tril_t[:].unsqueeze(1).to_broadcast((C, P, C)),
            )

        def s_update(p, t):
            ci = 2 * p + t
            phi = phi_bufs[p % 4]
            v = vx(ci)
            for j in range(P):
                nc.tensor.matmul(
                    s_psum[0:64, j * M1 : (j + 1) * M1],
                    phi[:, t, P * E + j * E : P * E + (j + 1) * E],
                    v[:, j * M1 : (j + 1) * M1],
                    start=(ci == 0 and j == 0),
                    stop=(ci == NCH - 1 and j == P - 1),
                    skip_group_check=True,
                )

        def inter(p, t):
            # Q @ S (first half of the out accumulation group)
            ci = 2 * p + t
            ops = out_ps[:, t, 0 : P * M1]
            sprev = ssb_bufs[(ci - 1) % 2]
            for j in range(P):
                nc.tensor.matmul(
                    ops[:, j * M1 : (j + 1) * M1],
                    qT(p, t, j),
                    sprev[:, j * M1 : (j + 1) * M1],
                    start=(j == 0), stop=False,
                )

        def intra(p, t):
            ci = 2 * p + t
            asb = attn_bufs[p % 2]
            ops = out_ps[:, t, 0 : P * M1]
            v = vx(ci)
            for j in range(P):
                nc.tensor.matmul(
                    ops[:, j * M1 : (j + 1) * M1],
                    asb[:, t, j * C : (j + 1) * C],
                    v[:, j * M1 : (j + 1) * M1],
                    start=(ci == 0 and j == 0), stop=(j == P - 1),
                )

        def s_evac(ci):
            if ci < NCH - 1:
                nc.scalar.activation(
                    ssb_bufs[ci % 2][:], s_psum[0:64, 0 : P * M1], AF.Copy
                )

        def out_evac(m):
            # both chunks of pair m in one op
            nc.scalar.activation(
                osb_bufs[(m // 2) % 2][:, (m % 2) * 2 : (m % 2) * 2 + 2, :],
                out_ps[:, :, 0 : P * M1], AF.Copy,
            )

        # ---- prologue: first load split in half so exp(0) starts early;
        # second load issued on the scalar ring in parallel ----
        load(0)
        nc.scalar.dma_start(out=qkv_bufs[1][:], in_=qkvr[1])
        exp_(0)
        max_(0)
        phi_(0)
        exp_(1)
        max_(1)

        # ---- pipeline: iter b: front f=b (transp/evac/attn), phi p2=b+1,
        # exp/max p3=b+2, mask+middle m=b-1, store group (b-2)//2 ----
        # distance-2 pipeline: every PE dependency resolves in a previous
        # iteration (mask is the only same-iteration PE gate and runs first
        # on DVE), so the PE queue drains nearly back-to-back.
        for b in range(NPAIR + 2):
            f, a, m, p2, p3 = b, b - 1, b - 2, b + 1, b + 2
            if 0 <= m < NPAIR:
                mask(2 * m)
                mask(2 * m + 1)
            if f < NPAIR:
                transp(f, 0)
                transp(f, 1)
            if b % 2 == 0 and b >= 2 and b // 2 + 1 < NG:
                exp_(b // 2 + 1)
            if 0 <= m < NPAIR:
                s_update(m, 0)
                if 2 * m > 0:
                    inter(m, 0)
                intra(m, 0)
                s_evac(2 * m)
            if f < NPAIR:
                tqk_evac(f, 0)
            if b % 2 == 0 and b >= 2 and b // 2 + 1 < NG:
                max_(b // 2 + 1)
            if 0 <= m < NPAIR:
                s_update(m, 1)
                inter(m, 1)
                s_evac(2 * m + 1)
                intra(m, 1)
            if 0 <= a < NPAIR:
                attn(a, 0)
            if f < NPAIR:
                tqk_evac(f, 1)
            if p2 < NPAIR:
                phi_(p2)
            if 0 <= a < NPAIR:
                attn(a, 1)
            if 0 <= m < NPAIR:
                out_evac(m)
            if b % 2 == 0 and b // 2 + 2 < NG:
                load(b // 2 + 2)
            if b % 2 == 1 and b >= 3:
                store((b - 3) // 2)

    return nc


def _tril_mask():
    # keep d<=c in [d,c] layout
    return np.triu(np.ones((C, C), np.float32)).astype(ml_dtypes.bfloat16)


def _ident_bf16():
    return np.eye(C, dtype=ml_dtypes.bfloat16)


def core_input_maps(queries, keys, values, key_lengths):
    """Build the 8 per-core input maps (host-side bf16 cast + qkv fusion)."""
    queries = np.asarray(queries, np.float32)
    keys = np.asarray(keys, np.float32)
    values = np.asarray(values, np.float32)
    key_lengths = np.asarray(key_lengths, np.float32)
    tril = _tril_mask()
    ident = _ident_bf16()
    maps = []
    for c in range(N_CORES):
        n, hg = c // 2, (c % 2) * P
        q = queries[n, :, hg : hg + P, :].reshape(L, P * E)
        k = keys[n, :, hg : hg + P, :].reshape(L, P * E)
        kl = key_lengths[n]
        vp = np.empty((L, P, M1), np.float32)
        vp[:, :, :E] = values[n, :, hg : hg + P, :] * kl[:, None, None]
        vp[:, :, E] = kl[:, None]
        qkv = np.concatenate([q, k, vp.reshape(L, P * M1)], axis=1)
        maps.append(
            {
                "qkv": np.ascontiguousarray(qkv.astype(ml_dtypes.bfloat16)),
                "tril": tril,
                "ident": ident,
            }
        )
    return maps


def assemble_output(results):
    """Gather per-core [num|den] rows and divide on the host (fp32)."""
    out = np.empty((N, L, H, E), np.float32)
    for c, r in enumerate(results):
        n, hg = c // 2, (c % 2) * P
        o = np.asarray(r["out"]).astype(np.float32).reshape(L, P, M1)
        out[n, :, hg : hg + P, :] = o[:, :, :E] / o[:, :, E : E + 1]
    return out


_CACHE = {}


def _get_nc():
    if "nc" not in _CACHE:
        nc = bacc.Bacc("TRN2", target_bir_lowering=False, debug=False)
        build_core_kernel(nc)
        nc.compile()
        _CACHE["nc"] = nc
    return _CACHE["nc"]


def kernel(queries, keys, values, key_lengths):
    nc = _get_nc()
    in_maps = core_input_maps(queries, keys, values, key_lengths)
    res = run_bass_kernel_spmd(nc, in_maps, list(range(N_CORES)))
    return assemble_output(res.results)


# revision 36
# speedup vs baseline: 1.1814x; 1.1814x over previous
"""Causal linear attention (fast-transformers style) on 8 Trainium2 NeuronCores.

Full inputs in, full output out. Sharding: the 32 (n, h) pairs are split
8 ways -> each core owns 4 pairs (one batch n, 4 adjacent heads); the
per-(n,h) cumulative KV state never crosses cores (no collectives).

v3 design (measured-rate driven; baseline was 114us, v2 104us):
- Host pre-casts to bf16 and fuses q|k|v' into ONE dram tensor per core
  (v' = [v*kl, kl] per pair: key_lengths folded into the value/ones
  columns on the host; the causal mask is then a plain multiply).
- 4-chunk DMA loads / stores; host does the final num/den divide.
- PE transposes via is_transpose -> bf16 PSUM (evac at DVE 2x rate).
- Uniform padded layout [q0|Z][q1|Z][q2|Z][q3|Z]: one 3-dim strided STT
  covers all four q blocks of a chunk; the S state lives at partitions
  0:64 for every pair, with the upper half of the s_sb tiles zeroed once
  so the 128-partition inter matmuls read zeros there.
- Software pipeline tuned against the measured loop-carried chain:
  the DVE queue runs [mask(2m), tq-evac(f), max, mask(2m+1), phi] so the
  attn->mask->...->tq->attn cycle is short; PE runs S-updates first
  (unblocks s_evac), staggers attn per chunk (attn(f,0) mid-iteration so
  mask(2f) next iteration has slack).
- Engine split: ACT {exp, tk-evac, s_evac, out-evac}, DVE {masks,
  tq-evac, max, phi}, PE {matmuls}. gpsimd only memsets (measured 40x
  slower than DVE on bulk elementwise).

Per-core math (chunked scan, C=128 rows, pairs j=0..3):
  phi(x) = elu(x)+1 = max(x,0) + min(exp(x),1)      (exact identity)
  per chunk, per pair j:
    attn_T[d,c] = sum_e K[d,e] Q[c,e], masked to d<=c (triu in [d,c])
    out = attn_T^T @ V' + Q @ S        (V' = [v*kl, kl], 65 cols)
    S  += K^T @ V'                     (PSUM accumulation)
  host: result = out[:, :64] / out[:, 64]
"""

from contextlib import ExitStack

import ml_dtypes
import numpy as np

import concourse.bacc as bacc
import concourse.mybir as mybir
import concourse.tile as tile
from concourse.bass_utils import run_bass_kernel_spmd

F32 = mybir.dt.float32
BF16 = mybir.dt.bfloat16
AF = mybir.ActivationFunctionType
ALU = mybir.AluOpType

N, L, H, E = 4, 4096, 8, 64
P = 4              # (n,h) pairs per core
C = 128            # chunk rows
M1 = E + 1         # v cols + kl column (denominator)
W = 3 * P * E + P  # fused qkv row: q 256 | k 256 | v' 260
N_CORES = 8
NCH = L // C       # 32 chunks
NPAIR = NCH // 2   # 16 chunk-pairs (pipeline iterations)
NG = NCH // 4      # 8 dma groups


def build_core_kernel(nc):
    qkv_d = nc.dram_tensor("qkv", [L, W], BF16, kind="ExternalInput").ap()
    tril_d = nc.dram_tensor("tril", [C, C], BF16, kind="ExternalInput").ap()
    ident_d = nc.dram_tensor("ident", [C, C], BF16, kind="ExternalInput").ap()
    out_d = nc.dram_tensor("out", [L, P * M1], BF16, kind="ExternalOutput").ap()

    qkvr = qkv_d.rearrange("(g t p) x -> g p t x", t=4, p=C)
    outr = out_d.rearrange("(g t p) x -> g p t x", t=4, p=C)

    with tile.TileContext(nc) as tc, ExitStack() as ctx:
        consts = ctx.enter_context(tc.tile_pool(name="consts", bufs=1))
        qkv_pool = ctx.enter_context(tc.tile_pool(name="qkv", bufs=1))
        e_pool = ctx.enter_context(tc.tile_pool(name="exp", bufs=1))
        x_pool = ctx.enter_context(tc.tile_pool(name="xmax", bufs=1))
        phi_pool = ctx.enter_context(tc.tile_pool(name="phi", bufs=1))
        qkT_pool = ctx.enter_context(tc.tile_pool(name="qkT", bufs=1))
        attn_pool = ctx.enter_context(tc.tile_pool(name="attn", bufs=1))
        osb_pool = ctx.enter_context(tc.tile_pool(name="osb", bufs=1))
        ssb_pool = ctx.enter_context(tc.tile_pool(name="ssb", bufs=1))
        ps_s = ctx.enter_context(tc.tile_pool(name="psS", bufs=1, space="PSUM"))
        ps_tq = ctx.enter_context(tc.tile_pool(name="psTq", bufs=1, space="PSUM"))
        ps_attn = ctx.enter_context(tc.tile_pool(name="psA", bufs=1, space="PSUM"))
        ps_out = ctx.enter_context(tc.tile_pool(name="psO", bufs=1, space="PSUM"))

        tril_t = consts.tile([C, C], BF16)
        nc.scalar.dma_start(out=tril_t[:], in_=tril_d[:])
        ident = consts.tile([C, C], BF16)
        nc.scalar.dma_start(out=ident[:], in_=ident_d[:])

        # rings (explicit tiles; slot = index % depth)
        qkv_bufs = [qkv_pool.tile([C, 4, W], BF16, name=f"qkv{i}") for i in range(4)]
        et_bufs = [e_pool.tile([C, 2, 2 * P * E], BF16, name=f"et{i}") for i in range(2)]
        xm_bufs = [x_pool.tile([C, 2, 2 * P * E], BF16, name=f"xm{i}") for i in range(2)]
        # phi: compact per chunk-pair [C, 2, 512] = [q0 q1 q2 q3 | k0 k1 k2 k3]
        # (no padding: matmul operands use 64-partition slices, all base 0)
        phi_bufs = [phi_pool.tile([C, 2, 4 * P * E // 2], BF16, name=f"phi{i}")
                    for i in range(4)]
        # transposed q+k: per-pair [64, 128] blocks, q at 0-3, k at 4-7
        qkT_bufs = [qkT_pool.tile([64, 2, 8 * C], BF16, name=f"qkT{i}") for i in range(3)]
        attn_bufs = [attn_pool.tile([C, 2, P * C], BF16, name=f"asb{i}") for i in range(2)]
        osb_bufs = [osb_pool.tile([C, 4, P * M1], BF16, name=f"osb{i}") for i in range(2)]
        ssb_bufs = [ssb_pool.tile([64, P * M1], BF16, name=f"ssb{i}") for i in range(2)]

        s_psum = ps_s.tile([C, 512], F32)
        # per-chunk transposed q+k tiles, ping-ponged by chunk parity
        tqk_ps = [ps_tq.tile([64, 8 * C], BF16, name=f"tqk{i}") for i in range(2)]
        # per-chunk attn tiles, ping-ponged by chunk parity
        attn_ps = [ps_attn.tile([C, P * C], F32, name=f"aps{i}") for i in range(2)]
        # one full bank per chunk so the j-slices never straddle a bank
        out_ps = ps_out.tile([C, 2, 512], F32)

        # ---- stage helpers (p = chunk-pair index, ci = chunk index) ----
        def qk_slice(p):
            return qkv_bufs[(p // 2) % 4][:, (p % 2) * 2 : (p % 2) * 2 + 2, 0 : 2 * P * E]

        def vx(ci):
            t = ci % 4
            return qkv_bufs[(ci // 4) % 4][:, t : t + 1, 2 * P * E : W].rearrange(
                "p a x -> p (a x)"
            )

        def load(g):
            nc.sync.dma_start(out=qkv_bufs[g % 4][:], in_=qkvr[g])

        def store(g):
            nc.sync.dma_start(out=outr[g], in_=osb_bufs[g % 2][:])

        def exp_(p):
            nc.scalar.activation(et_bufs[p % 2][:], qk_slice(p), AF.Exp)

        def max_(p):
            nc.vector.tensor_scalar_max(xm_bufs[p % 2][:], qk_slice(p), 0.0)

        def phi_(p):
            # phi = min(exp,1) + max(x,0): one fused STT (1x rate but a
            # single instruction and no extra cross-engine edges)
            nc.vector.scalar_tensor_tensor(
                phi_bufs[p % 4][:], et_bufs[p % 2][:], 1.0, xm_bufs[p % 2][:],
                op0=ALU.min, op1=ALU.add,
            )

        def transp(p, t):
            # per-pair [128, 64] -> [64, 128] transposes, all at base 0
            phi = phi_bufs[p % 4]
            for b in range(8):
                nc.tensor.transpose(
                    tqk_ps[t][:, b * C : (b + 1) * C],
                    phi[:, t, b * E : (b + 1) * E], ident[:],
                )

        def tqk_evac(p, t):
            dst = qkT_bufs[p % 3][:, t : t + 1, :].rearrange("p a x -> p (a x)")
            if t == 0:
                nc.vector.tensor_copy(dst, tqk_ps[t][:])
            else:
                nc.scalar.activation(dst, tqk_ps[t][:], AF.Copy)

        def qT(p, t, j):
            return qkT_bufs[p % 3][:, t, j * C : (j + 1) * C]

        def kT(p, t, j):
            return qkT_bufs[p % 3][:, t, (4 + j) * C : (5 + j) * C]

        def attn(p, t):
            ci = 2 * p + t
            for j in range(P):
                nc.tensor.matmul(
                    attn_ps[ci % 2][:, j * C : (j + 1) * C],
                    kT(p, t, j), qT(p, t, j),
                    start=(j == 0), stop=(j == P - 1),
                )

        def mask(ci):
            t = ci % 2
            nc.vector.tensor_mul(
                attn_bufs[(ci // 2) % 2][:, t : t + 1, :].rearrange(
                    "p a (j c) -> p (a j) c", c=C
                ),
                attn_ps[ci % 2][:].rearrange("p (j c) -> p j c", c=C),
                tril_t[:].unsqueeze(1).to_broadcast((C, P, C)),
            )

        def s_update(p, t):
            ci = 2 * p + t
            phi = phi_bufs[p % 4]
            v = vx(ci)
            for j in range(P):
                nc.tensor.matmul(
                    s_psum[0:64, j * M1 : (j + 1) * M1],
                    phi[:, t, P * E + j * E : P * E + (j + 1) * E],
                    v[:, j * M1 : (j + 1) * M1],
                    start=(ci == 0 and j == 0),
                    stop=(ci == NCH - 1 and j == P - 1),
                    skip_group_check=True,
                )

        def inter(p, t):
            # Q @ S (first half of the out accumulation group)
            ci = 2 * p + t
            ops = out_ps[:, t, 0 : P * M1]
            sprev = ssb_bufs[(ci - 1) % 2]
            for j in range(P):
                nc.tensor.matmul(
                    ops[:, j * M1 : (j + 1) * M1],
                    qT(p, t, j),
                    sprev[:, j * M1 : (j + 1) * M1],
                    start=(j == 0), stop=False,
                )

        def intra(p, t):
            ci = 2 * p + t
            asb = attn_bufs[p % 2]
            ops = out_ps[:, t, 0 : P * M1]
            v = vx(ci)
            for j in range(P):
                nc.tensor.matmul(
                    ops[:, j * M1 : (j + 1) * M1],
                    asb[:, t, j * C : (j + 1) * C],
                    v[:, j * M1 : (j + 1) * M1],
                    start=(ci == 0 and j == 0), stop=(j == P - 1),
                )

        def s_evac(ci):
            if ci < NCH - 1:
                nc.scalar.activation(
                    ssb_bufs[ci % 2][:], s_psum[0:64, 0 : P * M1], AF.Copy
                )

        def out_evac(m):
            # both chunks of pair m in one op
            nc.scalar.activation(
                osb_bufs[(m // 2) % 2][:, (m % 2) * 2 : (m % 2) * 2 + 2, :],
                out_ps[:, :, 0 : P * M1], AF.Copy,
            )

        # ---- prologue: first load split in half so exp(0) starts early;
        # second load issued on the scalar ring in parallel ----
        load(0)
        nc.scalar.dma_start(out=qkv_bufs[1][:], in_=qkvr[1])
        exp_(0)
        max_(0)
        phi_(0)
        exp_(1)
        max_(1)

        # ---- pipeline: iter b: front f=b (transp/evac/attn), phi p2=b+1,
        # exp/max p3=b+2, mask+middle m=b-1, store group (b-2)//2 ----
        # distance-2 pipeline: every PE dependency resolves in a previous
        # iteration (mask is the only same-iteration PE gate and runs first
        # on DVE), so the PE queue drains nearly back-to-back.
        for b in range(NPAIR + 2):
            f, a, m, p2, p3 = b, b - 1, b - 2, b + 1, b + 2
            if 0 <= m < NPAIR:
                mask(2 * m)
                mask(2 * m + 1)
            if f < NPAIR:
                transp(f, 0)
                transp(f, 1)
            if p3 < NPAIR:
                exp_(p3)
            if 0 <= m < NPAIR:
                s_update(m, 0)
                if 2 * m > 0:
                    inter(m, 0)
                intra(m, 0)
                s_evac(2 * m)
            if f < NPAIR:
                tqk_evac(f, 0)
            if p3 < NPAIR:
                max_(p3)
            if 0 <= m < NPAIR:
                s_update(m, 1)
                inter(m, 1)
                s_evac(2 * m + 1)
                intra(m, 1)
            if 0 <= a < NPAIR:
                attn(a, 0)
            if f < NPAIR:
                tqk_evac(f, 1)
            if p2 < NPAIR:
                phi_(p2)
            if 0 <= a < NPAIR:
                attn(a, 1)
            if 0 <= m < NPAIR:
                out_evac(m)
            if b % 2 == 0 and b // 2 + 2 < NG:
                load(b // 2 + 2)
            if b % 2 == 1 and b >= 3:
                store((b - 3) // 2)

    return nc


def _tril_mask():
    # keep d<=c in [d,c] layout
    return np.triu(np.ones((C, C), np.float32)).astype(ml_dtypes.bfloat16)


def _ident_bf16():
    return np.eye(C, dtype=ml_dtypes.bfloat16)


def core_input_maps(queries, keys, values, key_lengths):
    """Build the 8 per-core input maps (host-side bf16 cast + qkv fusion)."""
    queries = np.asarray(queries, np.float32)
    keys = np.asarray(keys, np.float32)
    values = np.asarray(values, np.float32)
    key_lengths = np.asarray(key_lengths, np.float32)
    tril = _tril_mask()
    ident = _ident_bf16()
    maps = []
    for c in range(N_CORES):
        n, hg = c // 2, (c % 2) * P
        q = queries[n, :, hg : hg + P, :].reshape(L, P * E)
        k = keys[n, :, hg : hg + P, :].reshape(L, P * E)
        kl = key_lengths[n]
        vp = np.empty((L, P, M1), np.float32)
        vp[:, :, :E] = values[n, :, hg : hg + P, :] * kl[:, None, None]
        vp[:, :, E] = kl[:, None]
        qkv = np.concatenate([q, k, vp.reshape(L, P * M1)], axis=1)
        maps.append(
            {
                "qkv": np.ascontiguousarray(qkv.astype(ml_dtypes.bfloat16)),
                "tril": tril,
                "ident": ident,
            }
        )
    return maps


def assemble_output(results):
    """Gather per-core [num|den] rows and divide on the host (fp32)."""
    out = np.empty((N, L, H, E), np.float32)
    for c, r in enumerate(results):
        n, hg = c // 2, (c % 2) * P
        o = np.asarray(r["out"]).astype(np.float32).reshape(L, P, M1)
        out[n, :, hg : hg + P, :] = o[:, :, :E] / o[:, :, E : E + 1]
    return out


_CACHE = {}


def _get_nc():
    if "nc" not in _CACHE:
        nc = bacc.Bacc("TRN2", target_bir_lowering=False, debug=False)
        build_core_kernel(nc)
        nc.compile()
        _CACHE["nc"] = nc
    return _CACHE["nc"]


def kernel(queries, keys, values, key_lengths):
    nc = _get_nc()
    in_maps = core_input_maps(queries, keys, values, key_lengths)
    res = run_bass_kernel_spmd(nc, in_maps, list(range(N_CORES)))
    return assemble_output(res.results)


# revision 37
# speedup vs baseline: 1.2683x; 1.0736x over previous
"""Causal linear attention (fast-transformers style) on 8 Trainium2 NeuronCores.

Full inputs in, full output out. Sharding: the 32 (n, h) pairs are split
8 ways -> each core owns 4 pairs (one batch n, 4 adjacent heads); the
per-(n,h) cumulative KV state never crosses cores (no collectives).

v3 design (measured-rate driven; baseline was 114us, v2 104us):
- Host pre-casts to bf16 and fuses q|k|v' into ONE dram tensor per core
  (v' = [v*kl, kl] per pair: key_lengths folded into the value/ones
  columns on the host; the causal mask is then a plain multiply).
- 4-chunk DMA loads / stores; host does the final num/den divide.
- PE transposes via is_transpose -> bf16 PSUM (evac at DVE 2x rate).
- Uniform padded layout [q0|Z][q1|Z][q2|Z][q3|Z]: one 3-dim strided STT
  covers all four q blocks of a chunk; the S state lives at partitions
  0:64 for every pair, with the upper half of the s_sb tiles zeroed once
  so the 128-partition inter matmuls read zeros there.
- Software pipeline tuned against the measured loop-carried chain:
  the DVE queue runs [mask(2m), tq-evac(f), max, mask(2m+1), phi] so the
  attn->mask->...->tq->attn cycle is short; PE runs S-updates first
  (unblocks s_evac), staggers attn per chunk (attn(f,0) mid-iteration so
  mask(2f) next iteration has slack).
- Engine split: ACT {exp, tk-evac, s_evac, out-evac}, DVE {masks,
  tq-evac, max, phi}, PE {matmuls}. gpsimd only memsets (measured 40x
  slower than DVE on bulk elementwise).

Per-core math (chunked scan, C=128 rows, pairs j=0..3):
  phi(x) = elu(x)+1 = max(x,0) + min(exp(x),1)      (exact identity)
  per chunk, per pair j:
    attn_T[d,c] = sum_e K[d,e] Q[c,e], masked to d<=c (triu in [d,c])
    out = attn_T^T @ V' + Q @ S        (V' = [v*kl, kl], 65 cols)
    S  += K^T @ V'                     (PSUM accumulation)
  host: result = out[:, :64] / out[:, 64]
"""

from contextlib import ExitStack

import ml_dtypes
import numpy as np

import concourse.bacc as bacc
import concourse.mybir as mybir
import concourse.tile as tile
from concourse.bass_utils import run_bass_kernel_spmd

F32 = mybir.dt.float32
BF16 = mybir.dt.bfloat16
AF = mybir.ActivationFunctionType
ALU = mybir.AluOpType

N, L, H, E = 4, 4096, 8, 64
P = 4              # (n,h) pairs per core
C = 128            # chunk rows
M1 = E + 1         # v cols + kl column (denominator)
W = 3 * P * E + P  # fused qkv row: q 256 | k 256 | v' 260
N_CORES = 8
NCH = L // C       # 32 chunks
NPAIR = NCH // 2   # 16 chunk-pairs (pipeline iterations)
NG = NCH // 4      # 8 dma groups


def build_core_kernel(nc):
    qkv_d = nc.dram_tensor("qkv", [L, W], BF16, kind="ExternalInput").ap()
    tril_d = nc.dram_tensor("tril", [C, C], BF16, kind="ExternalInput").ap()
    ident_d = nc.dram_tensor("ident", [C, C], BF16, kind="ExternalInput").ap()
    out_d = nc.dram_tensor("out", [L, P * M1], BF16, kind="ExternalOutput").ap()

    qkvr = qkv_d.rearrange("(g t p) x -> g p t x", t=4, p=C)
    outr = out_d.rearrange("(g t p) x -> g p t x", t=4, p=C)

    with tile.TileContext(nc) as tc, ExitStack() as ctx:
        consts = ctx.enter_context(tc.tile_pool(name="consts", bufs=1))
        qkv_pool = ctx.enter_context(tc.tile_pool(name="qkv", bufs=1))
        e_pool = ctx.enter_context(tc.tile_pool(name="exp", bufs=1))
        x_pool = ctx.enter_context(tc.tile_pool(name="xmax", bufs=1))
        phi_pool = ctx.enter_context(tc.tile_pool(name="phi", bufs=1))
        qkT_pool = ctx.enter_context(tc.tile_pool(name="qkT", bufs=1))
        attn_pool = ctx.enter_context(tc.tile_pool(name="attn", bufs=1))
        osb_pool = ctx.enter_context(tc.tile_pool(name="osb", bufs=1))
        ssb_pool = ctx.enter_context(tc.tile_pool(name="ssb", bufs=1))
        ps_s = ctx.enter_context(tc.tile_pool(name="psS", bufs=1, space="PSUM"))
        ps_tq = ctx.enter_context(tc.tile_pool(name="psTq", bufs=1, space="PSUM"))
        ps_attn = ctx.enter_context(tc.tile_pool(name="psA", bufs=1, space="PSUM"))
        ps_out = ctx.enter_context(tc.tile_pool(name="psO", bufs=1, space="PSUM"))

        tril_t = consts.tile([C, C], BF16)
        nc.scalar.dma_start(out=tril_t[:], in_=tril_d[:])
        ident = consts.tile([C, C], BF16)
        nc.scalar.dma_start(out=ident[:], in_=ident_d[:])

        # rings (explicit tiles; slot = index % depth)
        qkv_bufs = [qkv_pool.tile([C, 4, W], BF16, name=f"qkv{i}") for i in range(4)]
        et_bufs = [e_pool.tile([C, 2, 2 * P * E], BF16, name=f"et{i}") for i in range(2)]
        xm_bufs = [x_pool.tile([C, 2, 2 * P * E], BF16, name=f"xm{i}") for i in range(2)]
        # phi: compact per chunk-pair [C, 2, 512] = [q0 q1 q2 q3 | k0 k1 k2 k3]
        # (no padding: matmul operands use 64-partition slices, all base 0)
        phi_bufs = [phi_pool.tile([C, 2, 4 * P * E // 2], BF16, name=f"phi{i}")
                    for i in range(4)]
        # transposed q+k: per-pair [64, 128] blocks, q at 0-3, k at 4-7
        qkT_bufs = [qkT_pool.tile([64, 2, 8 * C], BF16, name=f"qkT{i}") for i in range(3)]
        attn_bufs = [attn_pool.tile([C, 2, P * C], BF16, name=f"asb{i}") for i in range(2)]
        osb_bufs = [osb_pool.tile([C, 4, P * M1], BF16, name=f"osb{i}") for i in range(2)]
        ssb_bufs = [ssb_pool.tile([64, P * M1], BF16, name=f"ssb{i}") for i in range(2)]

        s_psum = ps_s.tile([C, 512], F32)
        # per-chunk transposed q+k tiles, ping-ponged by chunk parity
        tqk_ps = [ps_tq.tile([64, 8 * C], BF16, name=f"tqk{i}") for i in range(2)]
        # per-chunk attn tiles, ping-ponged by chunk parity
        attn_ps = [ps_attn.tile([C, P * C], F32, name=f"aps{i}") for i in range(2)]
        # one full bank per chunk so the j-slices never straddle a bank
        out_ps = ps_out.tile([C, 2, 512], F32)

        # ---- stage helpers (p = chunk-pair index, ci = chunk index) ----
        def qk_slice(p):
            return qkv_bufs[(p // 2) % 4][:, (p % 2) * 2 : (p % 2) * 2 + 2, 0 : 2 * P * E]

        def vx(ci):
            t = ci % 4
            return qkv_bufs[(ci // 4) % 4][:, t : t + 1, 2 * P * E : W].rearrange(
                "p a x -> p (a x)"
            )

        def load(g):
            nc.sync.dma_start(out=qkv_bufs[g % 4][:], in_=qkvr[g])

        def store(g):
            nc.sync.dma_start(out=outr[g], in_=osb_bufs[g % 2][:])

        def exp_(p):
            nc.scalar.activation(et_bufs[p % 2][:], qk_slice(p), AF.Exp)

        def max_(p):
            nc.vector.tensor_scalar_max(xm_bufs[p % 2][:], qk_slice(p), 0.0)

        def phi_(p):
            # phi = min(exp,1) + max(x,0): one fused STT (1x rate but a
            # single instruction and no extra cross-engine edges)
            nc.vector.scalar_tensor_tensor(
                phi_bufs[p % 4][:], et_bufs[p % 2][:], 1.0, xm_bufs[p % 2][:],
                op0=ALU.min, op1=ALU.add,
            )

        def transp(p, t):
            # per-pair [128, 64] -> [64, 128] transposes, all at base 0
            phi = phi_bufs[p % 4]
            for b in range(8):
                nc.tensor.transpose(
                    tqk_ps[t][:, b * C : (b + 1) * C],
                    phi[:, t, b * E : (b + 1) * E], ident[:],
                )

        def tqk_evac(p, t):
            dst = qkT_bufs[p % 3][:, t : t + 1, :].rearrange("p a x -> p (a x)")
            if t == 0:
                nc.vector.tensor_copy(dst, tqk_ps[t][:])
            else:
                nc.scalar.activation(dst, tqk_ps[t][:], AF.Copy)

        def qT(p, t, j):
            return qkT_bufs[p % 3][:, t, j * C : (j + 1) * C]

        def kT(p, t, j):
            return qkT_bufs[p % 3][:, t, (4 + j) * C : (5 + j) * C]

        def attn(p, t):
            ci = 2 * p + t
            for j in range(P):
                nc.tensor.matmul(
                    attn_ps[ci % 2][:, j * C : (j + 1) * C],
                    kT(p, t, j), qT(p, t, j),
                    start=(j == 0), stop=(j == P - 1),
                )

        def mask(ci):
            t = ci % 2
            nc.vector.tensor_mul(
                attn_bufs[(ci // 2) % 2][:, t : t + 1, :].rearrange(
                    "p a (j c) -> p (a j) c", c=C
                ),
                attn_ps[ci % 2][:].rearrange("p (j c) -> p j c", c=C),
                tril_t[:].unsqueeze(1).to_broadcast((C, P, C)),
            )

        def s_update(p, t):
            ci = 2 * p + t
            phi = phi_bufs[p % 4]
            v = vx(ci)
            for j in range(P):
                nc.tensor.matmul(
                    s_psum[0:64, j * M1 : (j + 1) * M1],
                    phi[:, t, P * E + j * E : P * E + (j + 1) * E],
                    v[:, j * M1 : (j + 1) * M1],
                    start=(ci == 0 and j == 0),
                    stop=(ci == NCH - 1 and j == P - 1),
                    skip_group_check=True,
                )

        def inter(p, t):
            # Q @ S (first half of the out accumulation group)
            ci = 2 * p + t
            ops = out_ps[:, t, 0 : P * M1]
            sprev = ssb_bufs[(ci - 1) % 2]
            for j in range(P):
                nc.tensor.matmul(
                    ops[:, j * M1 : (j + 1) * M1],
                    qT(p, t, j),
                    sprev[:, j * M1 : (j + 1) * M1],
                    start=(j == 0), stop=False,
                )

        def intra(p, t):
            ci = 2 * p + t
            asb = attn_bufs[p % 2]
            ops = out_ps[:, t, 0 : P * M1]
            v = vx(ci)
            for j in range(P):
                nc.tensor.matmul(
                    ops[:, j * M1 : (j + 1) * M1],
                    asb[:, t, j * C : (j + 1) * C],
                    v[:, j * M1 : (j + 1) * M1],
                    start=(ci == 0 and j == 0), stop=(j == P - 1),
                )

        def s_evac(ci):
            if ci < NCH - 1:
                nc.scalar.activation(
                    ssb_bufs[ci % 2][:], s_psum[0:64, 0 : P * M1], AF.Copy
                )

        def out_evac(m):
            # both chunks of pair m in one op
            nc.scalar.activation(
                osb_bufs[(m // 2) % 2][:, (m % 2) * 2 : (m % 2) * 2 + 2, :],
                out_ps[:, :, 0 : P * M1], AF.Copy,
            )

        # ---- prologue: first load split in half so exp(0) starts early;
        # second load issued on the scalar ring in parallel ----
        load(0)
        load(1)
        exp_(0)
        max_(0)
        phi_(0)
        exp_(1)
        max_(1)

        # ---- pipeline: iter b: front f=b (transp/evac/attn), phi p2=b+1,
        # exp/max p3=b+2, mask+middle m=b-1, store group (b-2)//2 ----
        # distance-2 pipeline: every PE dependency resolves in a previous
        # iteration (mask is the only same-iteration PE gate and runs first
        # on DVE), so the PE queue drains nearly back-to-back.
        for b in range(NPAIR + 3):
            f, a, m, p2, p3 = b, b - 1, b - 2, b + 1, b + 2
            if 0 <= m < NPAIR:
                mask(2 * m)
                mask(2 * m + 1)
            if f < NPAIR:
                transp(f, 0)
                transp(f, 1)
            if p3 < NPAIR:
                exp_(p3)
            if 0 <= m < NPAIR:
                s_update(m, 0)
                if 2 * m > 0:
                    inter(m, 0)
                intra(m, 0)
                s_evac(2 * m)
            if f < NPAIR:
                tqk_evac(f, 0)
            if p3 < NPAIR:
                max_(p3)
            if 0 <= m < NPAIR:
                s_update(m, 1)
                inter(m, 1)
                s_evac(2 * m + 1)
                intra(m, 1)
            if 0 <= a < NPAIR:
                attn(a, 0)
            if f < NPAIR:
                tqk_evac(f, 1)
            if p2 < NPAIR:
                phi_(p2)
            if 0 <= a < NPAIR:
                attn(a, 1)
            if 0 <= m < NPAIR:
                out_evac(m)
            if b % 2 == 0 and b // 2 + 2 < NG:
                load(b // 2 + 2)
            if b % 2 == 0 and b >= 4:
                store((b - 4) // 2)

    return nc


def _tril_mask():
    # keep d<=c in [d,c] layout
    return np.triu(np.ones((C, C), np.float32)).astype(ml_dtypes.bfloat16)


def _ident_bf16():
    return np.eye(C, dtype=ml_dtypes.bfloat16)


def core_input_maps(queries, keys, values, key_lengths):
    """Build the 8 per-core input maps (host-side bf16 cast + qkv fusion)."""
    queries = np.asarray(queries, np.float32)
    keys = np.asarray(keys, np.float32)
    values = np.asarray(values, np.float32)
    key_lengths = np.asarray(key_lengths, np.float32)
    tril = _tril_mask()
    ident = _ident_bf16()
    maps = []
    for c in range(N_CORES):
        n, hg = c // 2, (c % 2) * P
        q = queries[n, :, hg : hg + P, :].reshape(L, P * E)
        k = keys[n, :, hg : hg + P, :].reshape(L, P * E)
        kl = key_lengths[n]
        vp = np.empty((L, P, M1), np.float32)
        vp[:, :, :E] = values[n, :, hg : hg + P, :] * kl[:, None, None]
        vp[:, :, E] = kl[:, None]
        qkv = np.concatenate([q, k, vp.reshape(L, P * M1)], axis=1)
        maps.append(
            {
                "qkv": np.ascontiguousarray(qkv.astype(ml_dtypes.bfloat16)),
                "tril": tril,
                "ident": ident,
            }
        )
    return maps


def assemble_output(results):
    """Gather per-core [num|den] rows and divide on the host (fp32)."""
    out = np.empty((N, L, H, E), np.float32)
    for c, r in enumerate(results):
        n, hg = c // 2, (c % 2) * P
        o = np.asarray(r["out"]).astype(np.float32).reshape(L, P, M1)
        out[n, :, hg : hg + P, :] = o[:, :, :E] / o[:, :, E : E + 1]
    return out


_CACHE = {}


def _get_nc():
    if "nc" not in _CACHE:
        nc = bacc.Bacc("TRN2", target_bir_lowering=False, debug=False)
        build_core_kernel(nc)
        nc.compile()
        _CACHE["nc"] = nc
    return _CACHE["nc"]


def kernel(queries, keys, values, key_lengths):
    nc = _get_nc()
    in_maps = core_input_maps(queries, keys, values, key_lengths)
    res = run_bass_kernel_spmd(nc, in_maps, list(range(N_CORES)))
    return assemble_output(res.results)
